# revision 4
# baseline (speedup 1.0000x reference)
"""GQA causal attention block (sparse_attention) on 8 Trainium2 NeuronCores.

Tensor-parallel over heads: core i computes q-heads 4i..4i+3 and kv-head i
(N_KV == n_cores), plus the matching row-slice of the o_proj; the 8 partial
o_proj outputs are summed on the host.

Performance model: the block is PE-bound, so the big GEMMs run as fp8-e4m3
DoubleRow matmuls (2 K-tiles per op at 0.5 cycles/row = 4x bf16 MACs):

  * q/k/v proj and o_proj use 3-term error compensation
    A@B ~= A8@B8 + dA8@B8 + A8@dB8  (dX8 = fp8 residual of X8), emitted as
    1 "main" DoubleRow op per chunk pair plus 1 "corr" op per chunk whose two
    slots carry (dA8 x B8) and (A8 x dB8) -- 0.75x the bf16 PE cycles with
    near-bf16 accuracy.  Every tensor is pre-scaled into e4m3's mid range
    (x*4, W*512, out*32) so base and residual both quantize well; the scales
    cancel in rms-norm or are divided out on the host.
  * scores stay bf16 (exp amplifies q/k error; bf16 is cheap enough here).
  * PV and the softmax denominator consume the fp8 exp output with two
    t-tiles paired per DoubleRow op (0.25x bf16), plus a v-residual term and,
    for the diagonal 512-block only (where the big softmax weights live), an
    exp-residual term -- measured end-to-end rel err ~2.4e-3 vs 2e-2 budget.

Layout: contraction dim on partitions everywhere; scores are computed
transposed [t, s]; softmax denominators via ones-matmul; reciprocals as
exp(-ln(x)) on ACT.  exp runs with a constant bias (scores max ~5.6, e4m3
max 240) that cancels exactly in the softmax ratio.
"""

import sys

sys.path.insert(0, "/opt/trn_rl_repo")

import numpy as np
import ml_dtypes

import concourse.bass as bass
import concourse.mybir as mybir
from concourse import tile
from concourse.vector_clock import ScopedClock, VectorClock
from concourse.bass_utils import run_bass_kernel_spmd

F32 = mybir.dt.float32
BF16 = mybir.dt.bfloat16
F8 = mybir.dt.float8e4
AF = mybir.ActivationFunctionType
OP = mybir.AluOpType
DR = mybir.MatmulPerfMode.DoubleRow

S = 2048
HID = 4096
N_HEADS = 32
N_KV = 8
D = 128
NCORES = 8
QH = N_HEADS // NCORES          # q heads per core
EPS = 1e-6
SM_SCALE = float(D) ** -0.5
NJ = S // 512                   # 512-wide s blocks
NHC = HID // 128                # 128-deep contraction chunks
NCP = NHC // 2                  # DoubleRow chunk pairs
NT = S // 128                   # 128-tall t tiles

# fp8 mid-range scaling
SX = 4.0                        # x -> x*SX
SWGT = 512.0                    # Wq/Wk/Wv/Wo -> W*SWGT
SV = 16.0                       # v stored at v*SV
SOUT = 32.0                     # attention out stored at out*SOUT
BIAS_C = 0.6                    # exp(s*SM_SCALE - BIAS_C); cancels in softmax
OSCALE = SOUT * SWGT            # o_proj psum scale, divided out on host
EPS_SC = EPS * (SX * SWGT) ** 2  # eps seen by rms-norm at psum scale
RD_BIAS = float(np.log(SOUT / SV))

f8np = ml_dtypes.float8_e4m3
bfnp = ml_dtypes.bfloat16


class TileContextFixed(tile.TileContext):
    """TileContext whose tail drain emits one sem-wait per Drain instruction.

    The pinned walrus (CoreV3GenImpl setupSyncWait) rejects instructions that
    carry more than one sync-wait command; stock TileContext attaches the
    whole global clock to a single Drain.
    """

    def _drain_and_barrier(self, tick_clock, wait_clock):
        gc = tick_clock.global_clock
        nprocs = len(gc)
        emitted = False
        for proc in range(nprocs):
            tick = gc[proc]
            if tick <= 0:
                continue
            vec = [0] * nprocs
            vec[proc] = tick
            d = self.nc.sync.drain()
            wait_clock.add_sem_waits(d.ins, ScopedClock({None: VectorClock(vec)}))
            emitted = True
        if not emitted:
            self.nc.sync.drain()

        self.nc.all_engine_barrier()
        assert self.sems is not None
        popped = self.nc._tile_sem_poison_stack.pop()
        assert popped is self._sem_poison
        self.nc.clear_and_free_semaphores(list(self.sems.allocated().values()))
        self.nc.all_engine_barrier()


def _split_multi_waits(nc):
    """Hoist all-but-one sem wait of any instruction onto preceding NOPs.

    The pinned walrus rejects instructions with more than one sync-wait
    command; engine streams execute in order, so a same-engine NOP carrying
    the extra waits right before the instruction is equivalent.
    """
    n = 0
    for f in nc.m.functions:
        for bb in f.blocks:
            rebuilt = []
            changed = False
            for inst in bb.instructions:
                si = inst.sync_info
                if si is not None and len(si.on_wait) > 1:
                    waits = list(si.on_wait)
                    for w in waits[:-1]:
                        n += 1
                        nop = mybir.InstNoOp(
                            name=f"I-waitsplit-{n}",
                            engine=inst.engine,
                            sync_info=mybir.SyncInfo(on_wait=[w], on_update=[]),
                            bass_nofuse=True,
                        )
                        nc.register_instruction(nop)
                        rebuilt.append(nop)
                    inst.sync_info = mybir.SyncInfo(
                        on_wait=[waits[-1]], on_update=list(si.on_update)
                    )
                    changed = True
                rebuilt.append(inst)
            if changed:
                bb.instructions = rebuilt


def build_program():
    nc = bass.Bass()

    # [j, cp, part, i, slot, col]: slot 0 = dx8, slot 1 = x8 (chunk 2cp+i)
    xt8 = nc.dram_tensor("xt8", [NJ, NCP, D, 2, 2, 512], F8, kind="ExternalInput")
    # [cp, part, i, slot, col]: slot 0 = W8, slot 1 = dW8; cols = 4*D q | D k | D v
    wqkv8 = nc.dram_tensor("wqkv8", [NCP, D, 2, 2, (QH + 2) * D], F8,
                           kind="ExternalInput")
    # [d, h, slot, col]: slot 0 = Wo8, slot 1 = dWo8
    wo8 = nc.dram_tensor("wo8", [D, QH, 2, HID], F8, kind="ExternalInput")
    # packed rope tables: [:, 0, :] = cos*w; [:, 1, :] = half-swapped rotate
    tabq = nc.dram_tensor("tabq", [D, 2, S], F32, kind="ExternalInput")
    tabk = nc.dram_tensor("tabk", [D, 2, S], F32, kind="ExternalInput")
    maskt = nc.dram_tensor("maskt", [16, D, 128], BF16, kind="ExternalInput")
    identb = nc.dram_tensor("identb", [D, D], BF16, kind="ExternalInput")
    ident8b = nc.dram_tensor("ident8b", [D, D], F8, kind="ExternalInput")
    out = nc.dram_tensor("out", [S, HID], F32, kind="ExternalOutput")

    with TileContextFixed(nc) as tc:
        with (
            tc.tile_pool(name="const", bufs=1) as constp,
            tc.tile_pool(name="persist", bufs=1) as persist,
            tc.tile_pool(name="wstream", bufs=6) as wstream,
            tc.tile_pool(name="xstream", bufs=6) as xstream,
            tc.tile_pool(name="tmp", bufs=2) as tmp,
            tc.tile_pool(name="tabstream", bufs=5) as tabstream,
            tc.tile_pool(name="expp", bufs=6) as expp,
            tc.tile_pool(name="outsb", bufs=2) as outsb,
            tc.tile_pool(name="ps", bufs=8, space="PSUM") as ps,
        ):
            ident = constp.tile([D, D], BF16, tag="ident")
            nc.gpsimd.dma_start(ident[:], identb[:])
            ident8 = constp.tile([D, D], F8, tag="ident8")
            nc.gpsimd.dma_start(ident8[:], ident8b[:])
            ones = constp.tile([D, D], BF16, tag="ones")
            nc.vector.memset(ones[:], 1.0)
            ones8 = constp.tile([D, 2, D], F8, tag="ones8")
            nc.vector.memset(ones8[:], 1.0)
            epsb = constp.tile([D, 1], F32, tag="epsb")
            nc.vector.memset(epsb[:], EPS_SC)
            cbias = constp.tile([D, 1], F32, tag="cbias")
            nc.vector.memset(cbias[:], -BIAS_C)
            rdbias = constp.tile([D, 1], F32, tag="rdbias")
            nc.vector.memset(rdbias[:], RD_BIAS)

            masks = persist.tile([D, 16, 128], BF16, tag="masks")
            wosb8 = persist.tile([D, QH, 2, HID], F8, tag="wosb8")

            # warm-up matmuls: keep the PE busy during the cold DMA ramp so
            # the HAM clock gate opens before the first projection matmuls
            pwarm = ps.tile([D, 512], F32, tag="ps", name="pwarm")
            for _w in range(24):
                nc.tensor.matmul(pwarm[:, 0:D], ones[:], ones[:],
                                 start=(_w == 0), stop=(_w == 23))

            qhat = [persist.tile([D, S], BF16, tag=f"qhat{h}", name=f"qhat{h}")
                    for h in range(QH)]
            khat = persist.tile([D, S], BF16, tag="khat")
            vsb8 = persist.tile([D, NT // 2, 2, D], F8, tag="vsb8")
            dvsb8 = persist.tile([D, NT // 2, 2, D], F8, tag="dvsb8")
            # [d, h, slot, s]: slot 0 = dout8, slot 1 = out8
            obig = persist.tile([D, QH, 2, S], F8, tag="obig")

            # dedicated diag-pair exp tiles, double-buffered by head parity.
            # Their beyond-diagonal slot-1 regions are zeroed once here and
            # never written again (exp writes only the valid window), so the
            # paired DoubleRow PV/den ops read zeros there.
            exA = persist.tile([D, 2, 2, 512], F8, tag="exA")
            exB = persist.tile([D, 2, 2, 512], F8, tag="exB")
            deA = persist.tile([D, 2, 2, 512], F8, tag="deA")
            deB = persist.tile([D, 2, 2, 512], F8, tag="deB")
            for hb in range(2):
                nc.vector.memset(exA[:, hb, 1, 0:128], 0.0)
                nc.vector.memset(deA[:, hb, 1, 0:128], 0.0)
                nc.vector.memset(exB[:, hb, 1, 256:384], 0.0)
                nc.vector.memset(deB[:, hb, 1, 256:384], 0.0)

            def emit_proj(j):
                """fp8 comp projections for s block j + PSUM evictions."""
                pq = [ps.tile([D, 512], F32, tag="ps", name=f"pq{_h}")
                      for _h in range(QH)]
                pk = ps.tile([D, 512], F32, tag="ps", name="pk")
                pv = ps.tile([D, 512], F32, tag="ps", name="pv")
                accs = pq + [pk, pv]
                for cp in range(NCP):
                    xt_t = xstream.tile([D, 2, 2, 512], F8, tag="xt", name="xt_t")
                    nc.sync.dma_start(xt_t[:], xt8[j, cp])
                    w_t = wstream.tile([D, 2, 2, (QH + 2) * D], F8, tag="w",
                                       name="w_t")
                    nc.scalar.dma_start(w_t[:], wqkv8[cp])
                    first = cp == 0
                    last = cp == NCP - 1
                    for o in range(QH + 2):
                        cols = slice(128 * o, 128 * (o + 1))
                        # main: (x8[2cp] W8[2cp]) + (x8[2cp+1] W8[2cp+1])
                        nc.tensor.matmul(accs[o][:], w_t[:, :, 0, cols],
                                         xt_t[:, :, 1, :], start=first,
                                         stop=False, perf_mode=DR)
                        # corr i: (dx8[c] W8[c]) + (x8[c] dW8[c])
                        for i in range(2):
                            nc.tensor.matmul(accs[o][:], w_t[:, i, :, cols],
                                             xt_t[:, i, :, :], start=False,
                                             stop=last and i == 1, perf_mode=DR)

                # evict accumulators right away to free the banks
                qraws = []
                for h in [QH] + list(range(QH)):
                    psrc = pk if h == QH else pq[h]
                    qraw = tmp.tile([D, 512], F32, tag="qraw", bufs=6, name="qraw")
                    nc.vector.tensor_copy(qraw[:], psrc[:])
                    sq = tmp.tile([D, 512], BF16, tag="sq", bufs=6, name="sq")
                    nc.vector.tensor_tensor(sq[:], qraw[:], qraw[:], OP.mult)
                    qraws.append((h, qraw, sq))
                # v: quantize wide to fp8 (+ residual) at scale SV
                v8w = tmp.tile([D, 512], F8, tag="v8w", name="v8w")
                nc.scalar.activation(v8w[:], pv[:], AF.Copy, bias=0.0,
                                     scale=SV / (SX * SWGT))
                tvw = tmp.tile([D, 512], BF16, tag="tvw", name="tvw")
                nc.scalar.activation(tvw[:], pv[:], AF.Copy, bias=0.0,
                                     scale=SV / (SX * SWGT))
                dv8w = tmp.tile([D, 512], F8, tag="dv8w", name="dv8w")
                nc.vector.tensor_tensor(dv8w[:], tvw[:], v8w[:], OP.subtract)
                return qraws, v8w, dv8w

            def emit_rope(j, qraws, v8w, dv8w):
                """RMS-norm + rope (k first) + fp8 v transposes for block j."""
                js = slice(512 * j, 512 * (j + 1))
                for h, qraw, sq in qraws:
                    if h < QH:
                        dstt, tdram = qhat[h], tabq
                    else:
                        dstt, tdram = khat, tabk
                    tab = tabstream.tile([D, 2, 512], F32, tag="tab", name="tab")
                    nc.sync.dma_start(tab[:], tdram[:, :, js])
                    pss = ps.tile([D, 512], F32, tag="ps", name="pss")
                    nc.tensor.matmul(pss[:], ones[:], sq[:], start=True, stop=True)
                    # r = rsqrt(mean + eps) = exp(-0.5 * ln(sumsq/128 + eps))
                    rbc = tmp.tile([D, 512], F32, tag="rbc", name="rbc")
                    nc.scalar.activation(rbc[:], pss[:], AF.Ln,
                                         bias=epsb[:], scale=1.0 / D)
                    nc.scalar.activation(rbc[:], rbc[:], AF.Exp, bias=0.0, scale=-0.5)
                    t1 = tmp.tile([D, 512], F32, tag="t1", name="t1")
                    nc.vector.tensor_tensor(t1[:], qraw[:], tab[:, 0, :], OP.mult)
                    t2 = tmp.tile([D, 512], F32, tag="t2", name="t2")
                    nc.vector.tensor_tensor(t2[0:64, :], qraw[64:128, :],
                                            tab[64:128, 1, :], OP.mult)
                    nc.vector.tensor_tensor(t2[64:128, :], qraw[0:64, :],
                                            tab[0:64, 1, :], OP.mult)
                    nc.vector.tensor_tensor(t1[:], t1[:], t2[:], OP.add)
                    nc.vector.tensor_tensor(dstt[:, js], t1[:], rbc[:], OP.mult)

                for c in range(4):
                    tt = 4 * j + c
                    p, sl = tt // 2, tt % 2
                    pvt = ps.tile([D, D, 2], F8, tag="ps", name="pvt")
                    nc.tensor.transpose(pvt[:, :, 0], v8w[:, 128 * c:128 * (c + 1)],
                                        ident8[:])
                    nc.scalar.copy(vsb8[:, p, sl, :], pvt[:, :, 0])
                    pdvt = ps.tile([D, D, 2], F8, tag="ps", name="pdvt")
                    nc.tensor.transpose(pdvt[:, :, 0],
                                        dv8w[:, 128 * c:128 * (c + 1)], ident8[:])
                    nc.scalar.copy(dvsb8[:, p, sl, :], pdvt[:, :, 0])

            def emit_attention(j):
                """Attention + o_proj for s block j (k/v tiles 0..4j+3 ready)."""
                js = slice(512 * j, 512 * (j + 1))
                npair = 2 * j + 2
                for h in range(QH):
                    hb = h % 2
                    po = ps.tile([D, 512], F32, tag="ps", name="po")
                    pd = ps.tile([D, 512], F32, tag="ps", name="pd")
                    pending = []

                    def flush_pair(ent, last):
                        p, cs, ext, det = ent
                        first = p == 0
                        # PV main + v-residual (+ diag exp-residual)
                        nc.tensor.matmul(po[:, cs], vsb8[:, p, :, :], ext[:, :, cs],
                                         start=first, stop=False, perf_mode=DR)
                        nc.tensor.matmul(po[:, cs], dvsb8[:, p, :, :], ext[:, :, cs],
                                         start=False, stop=last and det is None,
                                         perf_mode=DR)
                        if det is not None:
                            nc.tensor.matmul(po[:, cs], vsb8[:, p, :, :],
                                             det[:, :, cs], start=False, stop=last,
                                             perf_mode=DR)
                        # denominator
                        nc.tensor.matmul(pd[:, cs], ones8[:], ext[:, :, cs],
                                         start=first, stop=last and det is None,
                                         perf_mode=DR)
                        if det is not None:
                            nc.tensor.matmul(pd[:, cs], ones8[:], det[:, :, cs],
                                             start=False, stop=last, perf_mode=DR)

                    for p in range(npair):
                        tt0 = 2 * p
                        diag = tt0 >= 4 * j
                        if not diag:
                            cs = slice(0, 512)
                            ext = expp.tile([D, 2, 512], F8, tag="exF", name="exF")
                            det = None
                        elif tt0 == 4 * j:
                            cs = slice(0, 512)
                            ext, det = exA[:, hb], deA[:, hb]
                        else:
                            cs = slice(256, 512)
                            ext, det = exB[:, hb], deB[:, hb]
                        for ti in range(2):
                            tt = tt0 + ti
                            c0t = max(0, 128 * (tt - 4 * j))
                            cst = slice(c0t, 512)
                            psc = ps.tile([D, 512], F32, tag="ps", name="psc")
                            dtile = tt >= 4 * j
                            nc.tensor.matmul(psc[:, cst],
                                             khat[:, 128 * tt:128 * (tt + 1)],
                                             qhat[h][:, 512 * j + c0t:512 * (j + 1)],
                                             start=True, stop=not dtile)
                            if dtile:
                                # triangular boundary chunk: psc += I.T @ maskT
                                nc.tensor.matmul(psc[:, c0t:c0t + 128], ident[:],
                                                 masks[:, tt, :],
                                                 start=False, stop=True)
                            if not diag:
                                nc.scalar.activation(ext[:, ti, cst], psc[:, cst],
                                                     AF.Exp, bias=cbias[:],
                                                     scale=SM_SCALE)
                            else:
                                tex = tmp.tile([D, 512], BF16, tag="tex", bufs=4,
                                               name="tex")
                                nc.scalar.activation(tex[:, cst], psc[:, cst],
                                                     AF.Exp, bias=cbias[:],
                                                     scale=SM_SCALE)
                                nc.scalar.copy(ext[:, ti, cst], tex[:, cst])
                                nc.vector.tensor_tensor(det[:, ti, cst],
                                                        tex[:, cst], ext[:, ti, cst],
                                                        OP.subtract)
                        pending.append((p, cs, ext, det))
                        if len(pending) > 2:
                            flush_pair(pending.pop(0), last=False)
                    while pending:
                        flush_pair(pending.pop(0), last=len(pending) == 0)

                    # rd = (SOUT/SV) / den, computed as exp(-ln(den) + bias)
                    rd = tmp.tile([D, 512], F32, tag="rd", name="rd")
                    nc.scalar.activation(rd[:], pd[:], AF.Ln, bias=0.0, scale=1.0)
                    nc.scalar.activation(rd[:], rd[:], AF.Exp, bias=rdbias[:],
                                         scale=-1.0)
                    tpo = tmp.tile([D, 512], F32, tag="tpo", name="tpo")
                    nc.vector.tensor_tensor(tpo[:], po[:], rd[:], OP.mult)
                    nc.scalar.copy(obig[:, h, 1, js], tpo[:])
                    nc.vector.tensor_tensor(obig[:, h, 0, js], tpo[:],
                                            obig[:, h, 1, js], OP.subtract)

                for stt in range(4 * j, 4 * j + 4):
                    ss = slice(128 * stt, 128 * (stt + 1))
                    for half in range(2):
                        pb = [ps.tile([D, 512], F32, tag="ps", name=f"pb{_b}")
                              for _b in range(4)]
                        for b in range(4):
                            col = 2048 * half + 512 * b
                            cols = slice(col, col + 512)
                            # main: heads paired (out8[2m] Wo8[2m]) + ...
                            for m in range(2):
                                nc.tensor.matmul(
                                    pb[b][:], obig[:, 2 * m:2 * m + 2, 1, ss],
                                    wosb8[:, 2 * m:2 * m + 2, 0, cols],
                                    start=(m == 0), stop=False, perf_mode=DR)
                            # corr: (dout8[h] Wo8[h]) + (out8[h] dWo8[h])
                            for h in range(QH):
                                nc.tensor.matmul(
                                    pb[b][:], obig[:, h, :, ss],
                                    wosb8[:, h, :, cols],
                                    start=False, stop=h == QH - 1, perf_mode=DR)
                        osb = outsb.tile([D, 2048], F32, tag="osb", name="osb")
                        for b in range(4):
                            if b % 2 == 0:
                                nc.scalar.copy(osb[:, 512 * b:512 * (b + 1)], pb[b][:])
                            else:
                                nc.vector.tensor_copy(osb[:, 512 * b:512 * (b + 1)],
                                                      pb[b][:])
                        nc.gpsimd.dma_start(out[ss, 2048 * half:2048 * (half + 1)],
                                            osb[:])

            # Software-pipeline by one block: the PE stream per block is
            # [proj(j) | attention(j-1)+o_proj(j-1) | norm matmuls(j)], so the
            # ACT/DVE rope + norm chains for block j drain while the PE runs
            # attention for block j-1, and vice versa.
            for j in range(NJ):
                qraws, v8w, dv8w = emit_proj(j)
                for c in range(4):
                    tt = 4 * j + c
                    nc.gpsimd.dma_start(masks[:, tt, :], maskt[tt])
                if j == 0:
                    nc.gpsimd.dma_start(wosb8[:], wo8[:])
                if j > 0:
                    emit_attention(j - 1)
                emit_rope(j, qraws, v8w, dv8w)
            emit_attention(NJ - 1)

    _split_multi_waits(nc)
    return nc


_NC_CACHE = None


def _get_program():
    global _NC_CACHE
    if _NC_CACHE is None:
        _NC_CACHE = build_program()
    return _NC_CACHE


def _rope_tables(cos_g, sin_g, w):
    """Pack [D, 2, S]: [:, 0] = cos_g.T * w[d]; [:, 1] = swS where
    swS[d, s] = sign(pair(d)) * sin_g[s, pair(d)] * w[d]."""
    half = D // 2
    cw = np.ascontiguousarray((cos_g * w[None, :]).T)
    swS = np.empty((D, S), np.float32)
    swS[:half, :] = (sin_g[:, half:] * w[:half][None, :]).T
    swS[half:, :] = -(sin_g[:, :half] * w[half:][None, :]).T
    return np.ascontiguousarray(np.stack([cw, swS], axis=1))  # [D, 2, S]


def _q8(a):
    return np.clip(a, -240.0, 240.0).astype(f8np)


def _q8_pair(a):
    """fp8 base + fp8 residual of a (already scaled)."""
    a8 = _q8(a)
    da8 = _q8(a - a8.astype(np.float32))
    return a8, da8


def kernel(x, position_ids, cos, sin, attn_mask, Wq, Wk, Wv, Wo, q_norm_w, k_norm_w):
    x = np.asarray(x, np.float32)
    position_ids = np.asarray(position_ids)
    cos_g = np.asarray(cos, np.float32)[position_ids]   # [S, D]
    sin_g = np.asarray(sin, np.float32)[position_ids]
    attn_mask = np.asarray(attn_mask, np.float32)
    Wq = np.asarray(Wq, np.float32)
    Wk = np.asarray(Wk, np.float32)
    Wv = np.asarray(Wv, np.float32)
    Wo = np.asarray(Wo, np.float32)
    qw = np.asarray(q_norm_w, np.float32)
    kw = np.asarray(k_norm_w, np.float32)

    # x8/dx8 packed [NJ, NCP, 128, i, slot(0=dx8, 1=x8), 512]
    xs = np.ascontiguousarray(x.T) * SX                     # [HID, S]
    x8, dx8 = _q8_pair(xs)
    xt8 = np.empty((NJ, NCP, D, 2, 2, 512), f8np)
    x8r = x8.reshape(NCP, 2, D, NJ, 512)
    dx8r = dx8.reshape(NCP, 2, D, NJ, 512)
    xt8[:, :, :, :, 1, :] = x8r.transpose(3, 0, 2, 1, 4)
    xt8[:, :, :, :, 0, :] = dx8r.transpose(3, 0, 2, 1, 4)

    tabq = _rope_tables(cos_g, sin_g, qw)
    tabk = _rope_tables(cos_g, sin_g, kw)

    # diagonal-band mask tiles of attn_mask.T
    mT = attn_mask.T
    maskt = np.empty((16, D, 128), np.float32)
    for tt in range(16):
        c0 = 512 * (tt // 4) + 128 * (tt % 4)
        maskt[tt] = mT[128 * tt:128 * (tt + 1), c0:c0 + 128]
    maskt = maskt.astype(bfnp)

    identb = np.eye(D).astype(bfnp)
    ident8b = np.eye(D).astype(f8np)

    in_maps = []
    for i in range(NCORES):
        wcat = np.concatenate([
            Wq[:, QH * D * i:QH * D * (i + 1)],
            Wk[:, D * i:D * (i + 1)],
            Wv[:, D * i:D * (i + 1)],
        ], axis=1) * SWGT                                   # [HID, 768]
        w8, dw8 = _q8_pair(wcat)
        wqkv8 = np.empty((NCP, D, 2, 2, (QH + 2) * D), f8np)
        wqkv8[:, :, :, 0, :] = w8.reshape(NCP, 2, D, -1).transpose(0, 2, 1, 3)
        wqkv8[:, :, :, 1, :] = dw8.reshape(NCP, 2, D, -1).transpose(0, 2, 1, 3)

        wos = Wo[QH * D * i:QH * D * (i + 1), :] * SWGT     # [512, HID]
        wo8m, dwo8m = _q8_pair(wos)
        wo8 = np.empty((D, QH, 2, HID), f8np)
        wo8[:, :, 0, :] = wo8m.reshape(QH, D, HID).transpose(1, 0, 2)
        wo8[:, :, 1, :] = dwo8m.reshape(QH, D, HID).transpose(1, 0, 2)

        in_maps.append({
            "xt8": xt8,
            "wqkv8": np.ascontiguousarray(wqkv8),
            "wo8": np.ascontiguousarray(wo8),
            "tabq": tabq, "tabk": tabk,
            "maskt": maskt,
            "identb": identb,
            "ident8b": ident8b,
        })

    nc = _get_program()
    res = run_bass_kernel_spmd(nc, in_maps, list(range(NCORES)))
    acc = np.zeros((S, HID), np.float32)
    for r in res.results:
        acc += r["out"]
    return acc * (1.0 / OSCALE)


# revision 5
# speedup vs baseline: 1.3364x; 1.3364x over previous
"""GQA causal attention block (sparse_attention) on 8 Trainium2 NeuronCores.

Tensor-parallel over heads: core i computes q-heads 4i..4i+3 and kv-head i
(N_KV == n_cores, so each core owns exactly one kv head), plus the matching
row-slice of the o_proj; the 8 partial o_proj outputs are summed on the host.

Layout choice: everything that feeds the PE keeps the contraction dim on
partitions. Projections produce qT/kT/vT [d, s] directly (stationary = weight
chunk, moving = xT), attention scores are computed transposed [t, s]
(stationary = kT slice, moving = qT), PV consumes v [t, d] (stationary) times
exp-scores [t, s] (moving), and o_proj consumes outT [d, s] as stationary.
Softmax denominators come from a ones-matmul (partition-dim reduction on PE,
result pre-broadcast across partitions); reciprocals/rsqrts are computed as
exp(-ln(x)) on the ACT engine to avoid the slow iterative DVE divide.
"""

import sys

sys.path.insert(0, "/opt/trn_rl_repo")

import numpy as np
import ml_dtypes

import concourse.bass as bass
import concourse.mybir as mybir
from concourse import tile
from concourse.vector_clock import ScopedClock, VectorClock
from concourse.bass_utils import run_bass_kernel_spmd

F32 = mybir.dt.float32
BF16 = mybir.dt.bfloat16
AF = mybir.ActivationFunctionType
OP = mybir.AluOpType

S = 2048
HID = 4096
N_HEADS = 32
N_KV = 8
D = 128
NCORES = 8
QH = N_HEADS // NCORES          # q heads per core
EPS = 1e-6
SM_SCALE = float(D) ** -0.5
NJ = S // 512                   # 512-wide s blocks
NHC = HID // 128                # 128-deep contraction chunks
NT = S // 128                   # 128-tall t tiles


class TileContextFixed(tile.TileContext):
    """TileContext whose tail drain emits one sem-wait per Drain instruction.

    The pinned walrus (CoreV3GenImpl setupSyncWait) rejects instructions that
    carry more than one sync-wait command; stock TileContext attaches the
    whole global clock to a single Drain.
    """

    def _drain_and_barrier(self, tick_clock, wait_clock):
        gc = tick_clock.global_clock
        nprocs = len(gc)
        emitted = False
        for proc in range(nprocs):
            tick = gc[proc]
            if tick <= 0:
                continue
            vec = [0] * nprocs
            vec[proc] = tick
            d = self.nc.sync.drain()
            wait_clock.add_sem_waits(d.ins, ScopedClock({None: VectorClock(vec)}))
            emitted = True
        if not emitted:
            self.nc.sync.drain()

        self.nc.all_engine_barrier()
        assert self.sems is not None
        popped = self.nc._tile_sem_poison_stack.pop()
        assert popped is self._sem_poison
        self.nc.clear_and_free_semaphores(list(self.sems.allocated().values()))
        self.nc.all_engine_barrier()


def _split_multi_waits(nc):
    """Hoist all-but-one sem wait of any instruction onto preceding NOPs.

    The pinned walrus rejects instructions with more than one sync-wait
    command; engine streams execute in order, so a same-engine NOP carrying
    the extra waits right before the instruction is equivalent.
    """
    n = 0
    for f in nc.m.functions:
        for bb in f.blocks:
            rebuilt = []
            changed = False
            for inst in bb.instructions:
                si = inst.sync_info
                if si is not None and len(si.on_wait) > 1:
                    waits = list(si.on_wait)
                    for w in waits[:-1]:
                        n += 1
                        nop = mybir.InstNoOp(
                            name=f"I-waitsplit-{n}",
                            engine=inst.engine,
                            sync_info=mybir.SyncInfo(on_wait=[w], on_update=[]),
                            bass_nofuse=True,
                        )
                        nc.register_instruction(nop)
                        rebuilt.append(nop)
                    inst.sync_info = mybir.SyncInfo(
                        on_wait=[waits[-1]], on_update=list(si.on_update)
                    )
                    changed = True
                rebuilt.append(inst)
            if changed:
                bb.instructions = rebuilt


def build_program():
    nc = bass.Bass()

    # pre-tiled on host: xt[j, hc] = x.T[128*hc:128*(hc+1), 512*j:512*(j+1)]
    # so every projection DMA is one contiguous 128 KB read
    xt = nc.dram_tensor("xt", [NJ, NHC, D, 512], BF16, kind="ExternalInput")
    # packed per-core projection weights: [HID, 4*D q | D k | D v]
    wqkv = nc.dram_tensor("wqkv", [HID, (QH + 2) * D], BF16, kind="ExternalInput")
    wo = nc.dram_tensor("wo", [QH * D, HID], BF16, kind="ExternalInput")
    # packed rope tables: [:, 0, :] = cos*w; [:, 1, :] = half-swapped rotate
    # table swS with swS[d] = sign(pair(d))*sin[pair(d)]*w[d], so that
    # rot-half multiplies read both SBUF operands at the same base partition
    tabq = nc.dram_tensor("tabq", [D, 2, S], F32, kind="ExternalInput")
    tabk = nc.dram_tensor("tabk", [D, 2, S], F32, kind="ExternalInput")
    maskt = nc.dram_tensor("maskt", [16, D, 512], BF16, kind="ExternalInput")
    identb = nc.dram_tensor("identb", [D, D], BF16, kind="ExternalInput")
    out = nc.dram_tensor("out", [S, HID], F32, kind="ExternalOutput")

    with TileContextFixed(nc) as tc:
        with (
            tc.tile_pool(name="const", bufs=1) as constp,
            tc.tile_pool(name="persist", bufs=1) as persist,
            tc.tile_pool(name="wstream", bufs=14) as wstream,
            tc.tile_pool(name="xstream", bufs=14) as xstream,
            tc.tile_pool(name="tmp", bufs=2) as tmp,
            tc.tile_pool(name="tabstream", bufs=6) as tabstream,
            tc.tile_pool(name="expp", bufs=6) as expp,
            tc.tile_pool(name="outsb", bufs=2) as outsb,
            tc.tile_pool(name="ps", bufs=8, space="PSUM") as ps,
        ):
            ident = constp.tile([D, D], BF16, tag="ident")
            nc.gpsimd.dma_start(ident[:], identb[:])
            ones = constp.tile([D, D], BF16, tag="ones")
            nc.vector.memset(ones[:], 1.0)
            epsb = constp.tile([D, 1], F32, tag="epsb")
            nc.vector.memset(epsb[:], EPS)

            masks = persist.tile([D, 16, 512], BF16, tag="masks")
            wosb = persist.tile([D, QH, HID], BF16, tag="wosb")

            # warm-up matmuls: keep the PE busy during the cold DMA ramp so
            # the HAM clock gate opens before the first projection matmuls
            pwarm = ps.tile([D, 512], F32, tag="ps", name="pwarm")
            for _w in range(24):
                nc.tensor.matmul(pwarm[:, 0:D], ones[:], ones[:],
                                 start=(_w == 0), stop=(_w == 23))

            qhat = [persist.tile([D, S], BF16, tag=f"qhat{h}", name=f"qhat{h}")
                    for h in range(QH)]
            khat = persist.tile([D, S], BF16, tag="khat")
            vsb = persist.tile([D, NT, D], BF16, tag="vsb")
            outt = [persist.tile([D, S], BF16, tag=f"outt{h}", name=f"outt{h}")
                    for h in range(QH)]

            def emit_proj(j):
                """Projections for s block j + immediate PSUM evictions.

                Returns the evicted raw projections (SBUF) for the rope stage.
                """
                js = slice(512 * j, 512 * (j + 1))
                pq = [ps.tile([D, 512], F32, tag="ps", name=f"pq{_h}")
                      for _h in range(QH)]
                pk = ps.tile([D, 512], F32, tag="ps", name="pk")
                pv = ps.tile([D, 512], F32, tag="ps", name="pv")
                for hc in range(NHC):
                    xt_t = xstream.tile([D, 512], BF16, tag="xt", name="xt_t")
                    nc.sync.dma_start(xt_t[:], xt[j, hc])
                    w_t = wstream.tile([D, (QH + 2) * D], BF16, tag="w", name="w_t")
                    nc.scalar.dma_start(w_t[:], wqkv[128 * hc:128 * (hc + 1), :])
                    st = dict(start=(hc == 0), stop=(hc == NHC - 1))
                    for h in range(QH):
                        nc.tensor.matmul(pq[h][:], w_t[:, 128 * h:128 * (h + 1)],
                                         xt_t[:], **st)
                    nc.tensor.matmul(pk[:], w_t[:, QH * D:(QH + 1) * D], xt_t[:], **st)
                    nc.tensor.matmul(pv[:], w_t[:, (QH + 1) * D:], xt_t[:], **st)

                # evict all six accumulators right away to free the banks
                qraws = []
                for h in [QH] + list(range(QH)):
                    psrc = pk if h == QH else pq[h]
                    qraw = tmp.tile([D, 512], F32, tag="qraw", bufs=6, name="qraw")
                    nc.vector.tensor_copy(qraw[:], psrc[:])
                    sq = tmp.tile([D, 512], BF16, tag="sq", bufs=6, name="sq")
                    nc.vector.tensor_tensor(sq[:], qraw[:], qraw[:], OP.mult)
                    qraws.append((h, qraw, sq))
                vt = tmp.tile([D, 512], BF16, tag="vt", name="vt")
                nc.vector.tensor_copy(vt[:], pv[:])
                return qraws, vt

            def emit_rope(j, qraws, vt):
                """RMS-norm + rope (k first) + v transpose for s block j."""
                js = slice(512 * j, 512 * (j + 1))
                for h, qraw, sq in qraws:
                    if h < QH:
                        dstt, tdram = qhat[h], tabq
                    else:
                        dstt, tdram = khat, tabk
                    tab = tabstream.tile([D, 2, 512], F32, tag="tab", name="tab")
                    nc.sync.dma_start(tab[:], tdram[:, :, js])
                    pss = ps.tile([D, 512], F32, tag="ps", name="pss")
                    nc.tensor.matmul(pss[:], ones[:], sq[:], start=True, stop=True)
                    # r = rsqrt(mean + eps) = exp(-0.5 * ln(sumsq/128 + eps))
                    rbc = tmp.tile([D, 512], F32, tag="rbc", name="rbc")
                    nc.scalar.activation(rbc[:], pss[:], AF.Ln,
                                         bias=epsb[:], scale=1.0 / D)
                    nc.scalar.activation(rbc[:], rbc[:], AF.Exp, bias=0.0, scale=-0.5)
                    t1 = tmp.tile([D, 512], F32, tag="t1", name="t1")
                    nc.vector.tensor_tensor(t1[:], qraw[:], tab[:, 0, :], OP.mult)
                    t2 = tmp.tile([D, 512], F32, tag="t2", name="t2")
                    nc.vector.tensor_tensor(t2[0:64, :], qraw[64:128, :],
                                            tab[64:128, 1, :], OP.mult)
                    nc.vector.tensor_tensor(t2[64:128, :], qraw[0:64, :],
                                            tab[0:64, 1, :], OP.mult)
                    nc.vector.tensor_tensor(t1[:], t1[:], t2[:], OP.add)
                    nc.vector.tensor_tensor(dstt[:, js], t1[:], rbc[:], OP.mult)

                for c in range(4):
                    pvt = ps.tile([D, D], BF16, tag="ps", name="pvt")
                    nc.tensor.transpose(pvt[:], vt[:, 128 * c:128 * (c + 1)], ident[:])
                    nc.scalar.copy(vsb[:, 4 * j + c, :], pvt[:])

            def emit_attention(j):
                """Attention + o_proj for s block j (k/v tiles 0..4j+3 ready)."""
                js = slice(512 * j, 512 * (j + 1))
                for h in range(QH):
                    po = ps.tile([D, 512], F32, tag="ps", name="po")
                    pd = ps.tile([D, 512], F32, tag="ps", name="pd")
                    ntt = 4 * j + 4
                    pending = []
                    for tt in range(ntt):
                        # columns below c0 of this t tile are fully masked —
                        # skip them in scores/exp/PV/DEN (causal structure)
                        c0 = max(0, 128 * (tt - 4 * j))
                        cs = slice(c0, 512)
                        psc = ps.tile([D, 512], F32, tag="ps", name="psc")
                        diag = tt >= 4 * j
                        nc.tensor.matmul(psc[:, cs],
                                         khat[:, 128 * tt:128 * (tt + 1)],
                                         qhat[h][:, 512 * j + c0:512 * (j + 1)],
                                         start=True, stop=not diag)
                        if diag:
                            # triangular boundary chunk: psc += I.T @ maskT
                            nc.tensor.matmul(psc[:, c0:c0 + 128], ident[:],
                                             masks[:, tt, c0:c0 + 128],
                                             start=False, stop=True)
                        ex = expp.tile([D, 512], BF16, tag="ex", name="ex")
                        nc.scalar.activation(ex[:, cs], psc[:, cs], AF.Exp,
                                             bias=0.0, scale=SM_SCALE)
                        pending.append((tt, ex, cs))
                        # keep the PE three score tiles ahead of the exp chain
                        if len(pending) > 3:
                            ptt, pex, pcs = pending.pop(0)
                            stf = dict(start=(ptt == 0), stop=(ptt == ntt - 1))
                            nc.tensor.matmul(po[:, pcs], vsb[:, ptt, :],
                                             pex[:, pcs], **stf)
                            nc.tensor.matmul(pd[:, pcs], ones[:], pex[:, pcs],
                                             **stf)
                    for ptt, pex, pcs in pending:
                        stf = dict(start=(ptt == 0), stop=(ptt == ntt - 1))
                        nc.tensor.matmul(po[:, pcs], vsb[:, ptt, :], pex[:, pcs],
                                         **stf)
                        nc.tensor.matmul(pd[:, pcs], ones[:], pex[:, pcs], **stf)
                    rd = tmp.tile([D, 512], F32, tag="rd", name="rd")
                    nc.scalar.activation(rd[:], pd[:], AF.Ln, bias=0.0, scale=1.0)
                    nc.scalar.activation(rd[:], rd[:], AF.Exp, bias=0.0, scale=-1.0)
                    nc.vector.tensor_tensor(outt[h][:, js], po[:], rd[:], OP.mult)

                for stt in range(4 * j, 4 * j + 4):
                    ss = slice(128 * stt, 128 * (stt + 1))
                    for half in range(2):
                        pb = [ps.tile([D, 512], F32, tag="ps", name=f"pb{_b}")
                              for _b in range(4)]
                        for h in range(QH):
                            for b in range(4):
                                col = 2048 * half + 512 * b
                                nc.tensor.matmul(pb[b][:], outt[h][:, ss],
                                                 wosb[:, h, col:col + 512],
                                                 start=(h == 0), stop=(h == QH - 1))
                        osb = outsb.tile([D, 2048], F32, tag="osb", name="osb")
                        for b in range(4):
                            if b % 2 == 0:
                                nc.scalar.copy(osb[:, 512 * b:512 * (b + 1)], pb[b][:])
                            else:
                                nc.vector.tensor_copy(osb[:, 512 * b:512 * (b + 1)],
                                                      pb[b][:])
                        nc.gpsimd.dma_start(out[ss, 2048 * half:2048 * (half + 1)],
                                            osb[:])

            # Software-pipeline by one block: the PE stream per block is
            # [proj(j) | attention(j-1)+o_proj(j-1) | norm matmuls(j)], so the
            # ACT/DVE rope + norm chains for block j drain while the PE runs
            # attention for block j-1, and vice versa.
            for j in range(NJ):
                qraws, vt = emit_proj(j)
                # spread the mask/wo loads thinly across the stream so the
                # big transfers never starve the xt/w feed
                for c in range(4):
                    tt = 4 * j + c
                    nc.gpsimd.dma_start(masks[:, tt, :], maskt[tt])
                if j == 0:
                    nc.gpsimd.dma_start(
                        wosb[:], wo[:].rearrange("(h p) f -> p h f", p=D))
                if j > 0:
                    emit_attention(j - 1)
                emit_rope(j, qraws, vt)
            emit_attention(NJ - 1)

    _split_multi_waits(nc)
    return nc


_NC_CACHE = None


def _get_program():
    global _NC_CACHE
    if _NC_CACHE is None:
        _NC_CACHE = build_program()
    return _NC_CACHE


def _rope_tables(cos_g, sin_g, w):
    """Pack [D, 2, S]: [:, 0] = cos_g.T * w[d]; [:, 1] = swS where
    swS[d, s] = sign(pair(d)) * sin_g[s, pair(d)] * w[d], i.e. the rotate
    table with halves pre-swapped so t2[lo] = qraw[hi] * swS[hi] etc."""
    half = D // 2
    cw = np.ascontiguousarray((cos_g * w[None, :]).T)
    swS = np.empty((D, S), np.float32)
    swS[:half, :] = (sin_g[:, half:] * w[:half][None, :]).T
    swS[half:, :] = -(sin_g[:, :half] * w[half:][None, :]).T
    return np.ascontiguousarray(np.stack([cw, swS], axis=1))  # [D, 2, S]


def kernel(x, position_ids, cos, sin, attn_mask, Wq, Wk, Wv, Wo, q_norm_w, k_norm_w):
    x = np.asarray(x, np.float32)
    position_ids = np.asarray(position_ids)
    cos_g = np.asarray(cos, np.float32)[position_ids]   # [S, D]
    sin_g = np.asarray(sin, np.float32)[position_ids]
    attn_mask = np.asarray(attn_mask, np.float32)
    Wq = np.asarray(Wq, np.float32)
    Wk = np.asarray(Wk, np.float32)
    Wv = np.asarray(Wv, np.float32)
    Wo = np.asarray(Wo, np.float32)
    qw = np.asarray(q_norm_w, np.float32)
    kw = np.asarray(k_norm_w, np.float32)

    bf = ml_dtypes.bfloat16
    xt = np.ascontiguousarray(
        x.T.reshape(NHC, D, NJ, 512).transpose(2, 0, 1, 3)).astype(bf)

    tabq = _rope_tables(cos_g, sin_g, qw)
    tabk = _rope_tables(cos_g, sin_g, kw)

    # diagonal-band mask tiles of attn_mask.T: tile tt covers scoresT rows
    # 128*tt..128*tt+127 and cols (q positions) 512*(tt//4)..+511
    mT = attn_mask.T
    maskt = np.empty((16, D, 512), np.float32)
    for tt in range(16):
        j = tt // 4
        maskt[tt] = mT[128 * tt:128 * (tt + 1), 512 * j:512 * (j + 1)]
    maskt = maskt.astype(ml_dtypes.bfloat16)

    identb = np.eye(D).astype(bf)

    in_maps = []
    for i in range(NCORES):
        wqkv = np.concatenate([
            Wq[:, QH * D * i:QH * D * (i + 1)],
            Wk[:, D * i:D * (i + 1)],
            Wv[:, D * i:D * (i + 1)],
        ], axis=1).astype(bf)
        in_maps.append({
            "xt": xt,
            "wqkv": np.ascontiguousarray(wqkv),
            "wo": np.ascontiguousarray(Wo[QH * D * i:QH * D * (i + 1), :]).astype(bf),
            "tabq": tabq, "tabk": tabk,
            "maskt": maskt,
            "identb": identb,
        })

    nc = _get_program()
    res = run_bass_kernel_spmd(nc, in_maps, list(range(NCORES)))
    acc = np.zeros((S, HID), np.float32)
    for r in res.results:
        acc += r["out"]
    return acc

